# revision 49
# baseline (speedup 1.0000x reference)
"""Expert-choice FF kernel for Trainium2 (Bass/Tile), 8-core expert-parallel.

The reference computes (per expert e of 32):
    xe = x.reshape(32, 256, 1024)[e]          # contiguous token groups
    out[e] = relu(xe @ lin1[e]) @ lin2[e]
(The gate/softmax/top_k in the reference are dead code w.r.t. the output.)

Sharding: expert-parallel, 4 experts per core across 8 cores. Each core runs
two chained GEMMs per expert with fp32 PSUM accumulation over bf16 operands.

Layout (host-prepared so every DMA is contiguous per partition):
    xT[e]  = (128 p, 8 k, 256 m)       : xT[e][p,k,m]   = xe[m, k*128+p]
    w1[e]  = (128 p, 4 q, 8 k, 512 c)  : w1[e][p,q,k,c] = lin1[e][k*128+p, q*512+c]
    w2[e]  = (128 p, 16 s, 1024 d)     : w2[e][p,s,d]   = lin2[e][s*128+p, d]
    out[e] = (128 p, 2 mo, 1024 d)     : out[e][p,mo,d] = out_e[mo*128+p, d]

GEMM1 computes hT (expert_size on partitions) so GEMM2 needs no transpose:
    hT[n,m] = relu(sum_k w1[k,n] * xT[k,m])      (psum [128n, 256m], 8 k-accum)
    out[m,d] = sum_s hT[s,m] * w2[s,d]           (psum [128m, 512d], 16 s-accum)
"""

import numpy as np
import ml_dtypes
from contextlib import ExitStack

N_EXPERTS = 32
TOPK = 256
DMODEL = 1024
EXPERT_SIZE = 2048
BATCH = 2
CUTOFF = 4096

N_CORES = 8
EPC = N_EXPERTS // N_CORES  # experts per core = 4
P = 128
KO = DMODEL // P        # 8 k-tiles (GEMM1 contraction)
SO = EXPERT_SIZE // P   # 16 s-tiles (GEMM2 contraction / GEMM1 output)
MO = TOPK // P          # 2 token chunks
DT = 512                # GEMM2 moving free dim (one psum bank of fp32)
ND = DMODEL // DT       # 2 d-tiles
NQ = 4                  # w1 n-quarters (GEMM1 consumption-ordered chunks)
QW = EXPERT_SIZE // NQ  # 512 n-cols per quarter

_CACHE = {}


def _build_nc():
    import concourse.mybir as mybir
    import concourse.tile as tile
    from concourse import bacc

    BF16 = mybir.dt.bfloat16
    FP32 = mybir.dt.float32
    RELU = mybir.ActivationFunctionType.Relu

    nc = bacc.Bacc("TRN2", debug=False, enable_asserts=False,
                   num_devices=N_CORES, enable_partition_id=False)
    xt_d = nc.dram_tensor("xt", (EPC, P, KO, TOPK), BF16,
                          kind="ExternalInput").ap()
    w1_d = nc.dram_tensor("w1", (EPC, P, NQ, KO, QW), BF16,
                          kind="ExternalInput").ap()
    w2_d = nc.dram_tensor("w2", (EPC, P, SO, DMODEL), BF16,
                          kind="ExternalInput").ap()
    out_d = nc.dram_tensor("out", (EPC, P, MO, DMODEL), FP32,
                           kind="ExternalOutput").ap()

    with tile.TileContext(nc, pool_alloc_mode="queue") as tc, ExitStack() as ctx:
        xp = ctx.enter_context(tc.tile_pool(name="xt", bufs=3))
        w1p = ctx.enter_context(tc.tile_pool(name="w1", bufs=2))
        w2p = ctx.enter_context(tc.tile_pool(name="w2", bufs=2))
        hp = ctx.enter_context(tc.tile_pool(name="ht", bufs=2))
        op = ctx.enter_context(tc.tile_pool(name="ot", bufs=2))
        wzp = ctx.enter_context(tc.tile_pool(name="wz", bufs=1))
        ps1 = ctx.enter_context(tc.tile_pool(name="ps1", bufs=6, space="PSUM"))
        ps2 = ctx.enter_context(tc.tile_pool(name="ps2", bufs=2, space="PSUM"))

        # PE warmup: dummy matmuls on zeroed SBUF while the first DMAs land,
        # so HAM un-throttles (1.2 -> 2.4 GHz) before real work — sized to
        # bridge all the way to the first weight chunk's arrival (~14us).
        wz = wzp.tile([P, TOPK], BF16, name="wz", tag="wz")
        nc.any.memzero(wz[:])
        pw = ps2.tile([P, DT], FP32, name="psw", tag="ps2")
        for i in range(30):
            nc.tensor.matmul(pw[:, :TOPK], lhsT=wz[:, :P], rhs=wz[:],
                             start=True, stop=True)

        for e in range(EPC):
            # Loads: all ~0.5-1MB and in consumption order — queues drain
            # round-robin at packet granularity, so uniform chunks keep
            # completion near-FIFO while staying DMA-efficient.
            xt = xp.tile([P, KO, TOPK], BF16, name=f"xt_{e}", tag="xt")
            w1t = w1p.tile([P, NQ, KO, QW], BF16, name=f"w1t_{e}", tag="w1")
            w2t = w2p.tile([P, SO, DMODEL], BF16, name=f"w2t_{e}", tag="w2")
            if e == 0:
                # Split the head of the stream extra-fine so the first
                # matmuls' data lands ASAP (HWDGE completion receipt adds
                # ~3us after the last byte, so small head chunks win).
                nc.sync.dma_start(xt[:, :KO // 2], xt_d[e, :, :KO // 2])
                nc.sync.dma_start(w1t[:, 0, :2], w1_d[e, :, 0, :2])
                nc.sync.dma_start(w1t[:, 0, 2:4], w1_d[e, :, 0, 2:4])
                nc.sync.dma_start(xt[:, KO // 2:], xt_d[e, :, KO // 2:])
                nc.sync.dma_start(w1t[:, 0, 4:6], w1_d[e, :, 0, 4:6])
                nc.sync.dma_start(w1t[:, 0, 6:], w1_d[e, :, 0, 6:])
                for q in range(1, NQ):
                    nc.sync.dma_start(w1t[:, q, :KO // 2],
                                      w1_d[e, :, q, :KO // 2])
                    nc.sync.dma_start(w1t[:, q, KO // 2:],
                                      w1_d[e, :, q, KO // 2:])
            else:
                # Steady-state experts are prefetched a full expert ahead,
                # so fewer/bigger DMAs win (less SP issue time, higher DMA
                # efficiency); round-robin completion skew is absorbed.
                nc.sync.dma_start(xt[:], xt_d[e])
                nc.sync.dma_start(w1t[:, :2], w1_d[e, :, :2])
                nc.sync.dma_start(w1t[:, 2:], w1_d[e, :, 2:])
            if e == 0:
                for s4 in range(SO // 4):
                    nc.sync.dma_start(w2t[:, 4 * s4:4 * s4 + 4],
                                      w2_d[e, :, 4 * s4:4 * s4 + 4])
            else:
                nc.sync.dma_start(w2t[:, :SO // 2], w2_d[e, :, :SO // 2])
                nc.sync.dma_start(w2t[:, SO // 2:], w2_d[e, :, SO // 2:])

            # GEMM1: hT = relu(w1.T @ xT), k-outer over 4 concurrent psum
            # groups per n-quarter (ps1 has 6 slots, so quarter boundaries
            # mostly find free slots and don't wait on the relu drain).
            ht = hp.tile([P, SO * TOPK], BF16, name=f"ht_{e}", tag="ht")
            for q in range(NQ):
                pts = [ps1.tile([P, TOPK], FP32,
                                name=f"ps1_{e}_{q}_{j}", tag="ps1")
                       for j in range(4)]
                for k in range(KO):
                    for j in range(4):
                        nc.tensor.matmul(
                            pts[j][:],
                            lhsT=w1t[:, q, k, j * P:(j + 1) * P],
                            rhs=xt[:, k],
                            start=(k == 0), stop=(k == KO - 1))
                for j in range(4):
                    n0 = q * 4 + j
                    nc.scalar.activation(
                        ht[:, n0 * TOPK:(n0 + 1) * TOPK], pts[j][:], RELU)

            # GEMM2: out = hT.T @ w2, group-sequential over (m,d): staggered
            # completion lets copies/stores overlap remaining matmuls; the
            # last expert stores on the (now idle) HWDGE rings to avoid the
            # SWDGE drain tail.
            ot = op.tile([P, MO, DMODEL], FP32, name=f"ot_{e}", tag="ot")
            last = e == EPC - 1
            for m in range(MO):
                for d in range(ND):
                    final = last and m == MO - 1 and d == ND - 1
                    # The very last group runs as two 256-wide half-groups
                    # (same PE cycles) so its first half's copy+store
                    # overlap the second half's matmuls — shorter tail.
                    widths = [TOPK, TOPK] if final else [DT]
                    off = d * DT
                    for w in widths:
                        pt2g = ps2.tile([P, DT], FP32,
                                        name=f"ps2_{e}_{m}_{d}_{off}",
                                        tag="ps2")
                        for s in range(SO):
                            nc.tensor.matmul(
                                pt2g[:, :w],
                                lhsT=ht[:, s * TOPK + m * P:
                                        s * TOPK + (m + 1) * P],
                                rhs=w2t[:, s, off:off + w],
                                start=(s == 0), stop=(s == SO - 1))
                        nc.vector.tensor_copy(
                            out=ot[:, m, off:off + w], in_=pt2g[:, :w])
                        if last:
                            dma_eng = nc.scalar if d == 0 else nc.sync
                            dma_eng.dma_start(
                                out_d[e, :, m, off:off + w],
                                ot[:, m, off:off + w])
                        off += w
            if not last:
                nc.gpsimd.dma_start(out_d[e], ot[:])

    nc.compile()
    return nc


def _get_nc():
    if "nc" not in _CACHE:
        _CACHE["nc"] = _build_nc()
    return _CACHE["nc"]


def _prep_inputs(x, lin1_weight, lin2_weight):
    bf16 = ml_dtypes.bfloat16
    x_flat = np.asarray(x, dtype=np.float32).reshape(N_EXPERTS * TOPK, DMODEL)
    # xt_all[g, p, ko, m] = x_flat[g*TOPK + m, ko*P + p]
    xt_all = np.ascontiguousarray(
        x_flat.reshape(N_EXPERTS, TOPK, KO, P).transpose(0, 3, 2, 1)
    ).astype(bf16)
    # w1_all[g, p, q, k, c] = lin1[g, k*P + p, q*QW + c]
    w1_all = np.ascontiguousarray(
        np.asarray(lin1_weight, dtype=np.float32)
        .reshape(N_EXPERTS, KO, P, NQ, QW).transpose(0, 2, 3, 1, 4)
    ).astype(bf16)
    # w2_all[g, p, so, d] = lin2[g, so*P + p, d]
    w2_all = np.ascontiguousarray(
        np.asarray(lin2_weight, dtype=np.float32)
        .reshape(N_EXPERTS, SO, P, DMODEL).transpose(0, 2, 1, 3)
    ).astype(bf16)
    in_maps = []
    for c in range(N_CORES):
        sl = slice(c * EPC, (c + 1) * EPC)
        in_maps.append({
            "xt": np.ascontiguousarray(xt_all[sl]),
            "w1": np.ascontiguousarray(w1_all[sl]),
            "w2": np.ascontiguousarray(w2_all[sl]),
        })
    return in_maps


def _ensure_axon_hooks():
    """bass_utils imports antenv.axon_hooks when tracing under axon; the RL
    image ships only an antenv stub. Provide the module (backed by the axon
    PJRT plugin's nrt-profile C ABI when present) so trace requests never
    crash the kernel."""
    try:
        import antenv.axon_hooks  # noqa: F401
        return
    except ImportError:
        pass
    import contextlib
    import ctypes
    import sys
    import types

    hook = None
    so_path = "/opt/axon/libaxon_pjrt.so"
    try:
        lib = ctypes.CDLL(so_path)
        if hasattr(lib, "axon_start_nrt_profile"):
            lib.axon_start_nrt_profile.argtypes = [
                ctypes.POINTER(ctypes.c_int64), ctypes.c_size_t]
            lib.axon_start_nrt_profile.restype = ctypes.c_int64
            lib.axon_stop_nrt_profile.argtypes = [ctypes.c_char_p]
            lib.axon_stop_nrt_profile.restype = ctypes.c_int64

            @contextlib.contextmanager
            def _hook(output_dir, device_ids):
                import jax
                jax.devices()
                if device_ids:
                    ids = (ctypes.c_int64 * len(device_ids))(*device_ids)
                    rc = lib.axon_start_nrt_profile(ids, len(device_ids))
                else:
                    rc = lib.axon_start_nrt_profile(None, 0)
                if rc != 0:
                    raise RuntimeError(f"axon_start_nrt_profile rc={rc}")
                try:
                    yield
                finally:
                    lib.axon_stop_nrt_profile(str(output_dir).encode())

            hook = _hook
    except OSError:
        pass

    mod = types.ModuleType("antenv.axon_hooks")
    mod.get_axon_ntff_profile_hook = lambda: hook
    mod.set_axon_ntff_profile_hook = lambda h: None
    sys.modules["antenv.axon_hooks"] = mod


def _run(in_maps, trace=False, **kw):
    _ensure_axon_hooks()
    from concourse.bass_utils import run_bass_kernel_spmd
    nc = _get_nc()
    return run_bass_kernel_spmd(nc, in_maps, core_ids=list(range(N_CORES)),
                                trace=trace, **kw)


def kernel(x, gate, lin1_weight, lin2_weight, _trace=False, _kw=None):
    in_maps = _prep_inputs(x, lin1_weight, lin2_weight)
    res = _run(in_maps, trace=_trace, **(_kw or {}))
    _CACHE["last_results"] = res
    # res.results[c]["out"]: (EPC, P, MO, DMODEL) fp32
    out_all = np.stack([res.results[c]["out"] for c in range(N_CORES)])
    out_all = out_all.reshape(N_EXPERTS, P, MO, DMODEL)
    # out_e[mo*P + p, d] = out_all[g, p, mo, d]
    out = out_all.transpose(0, 2, 1, 3).reshape(BATCH, CUTOFF, DMODEL)
    return np.ascontiguousarray(out)


# revision 50
# speedup vs baseline: 1.0438x; 1.0438x over previous
"""Expert-choice FF kernel for Trainium2 (Bass/Tile), 8-core expert-parallel.

The reference computes (per expert e of 32):
    xe = x.reshape(32, 256, 1024)[e]          # contiguous token groups
    out[e] = relu(xe @ lin1[e]) @ lin2[e]
(The gate/softmax/top_k in the reference are dead code w.r.t. the output.)

Sharding: expert-parallel, 4 experts per core across 8 cores. Each core runs
two chained GEMMs per expert with fp32 PSUM accumulation over bf16 operands.

Layout (host-prepared so every DMA is contiguous per partition):
    xT[e]  = (128 p, 8 k, 256 m)       : xT[e][p,k,m]   = xe[m, k*128+p]
    w1[e]  = (128 p, 4 q, 8 k, 512 c)  : w1[e][p,q,k,c] = lin1[e][k*128+p, q*512+c]
    w2[e]  = (128 p, 16 s, 1024 d)     : w2[e][p,s,d]   = lin2[e][s*128+p, d]
    out[e] = (128 p, 2 mo, 1024 d)     : out[e][p,mo,d] = out_e[mo*128+p, d]

GEMM1 computes hT (expert_size on partitions) so GEMM2 needs no transpose:
    hT[n,m] = relu(sum_k w1[k,n] * xT[k,m])      (psum [128n, 256m], 8 k-accum)
    out[m,d] = sum_s hT[s,m] * w2[s,d]           (psum [128m, 512d], 16 s-accum)
"""

import numpy as np
import ml_dtypes
from contextlib import ExitStack

N_EXPERTS = 32
TOPK = 256
DMODEL = 1024
EXPERT_SIZE = 2048
BATCH = 2
CUTOFF = 4096

N_CORES = 8
EPC = N_EXPERTS // N_CORES  # experts per core = 4
P = 128
KO = DMODEL // P        # 8 k-tiles (GEMM1 contraction)
SO = EXPERT_SIZE // P   # 16 s-tiles (GEMM2 contraction / GEMM1 output)
MO = TOPK // P          # 2 token chunks
DT = 512                # GEMM2 moving free dim (one psum bank of fp32)
ND = DMODEL // DT       # 2 d-tiles
NQ = 4                  # w1 n-quarters (GEMM1 consumption-ordered chunks)
QW = EXPERT_SIZE // NQ  # 512 n-cols per quarter

_CACHE = {}


def _build_nc():
    import concourse.mybir as mybir
    import concourse.tile as tile
    from concourse import bacc

    BF16 = mybir.dt.bfloat16
    FP32 = mybir.dt.float32
    RELU = mybir.ActivationFunctionType.Relu

    nc = bacc.Bacc("TRN2", debug=False, enable_asserts=False,
                   num_devices=N_CORES, enable_partition_id=False)
    xt_d = nc.dram_tensor("xt", (EPC, P, KO, TOPK), BF16,
                          kind="ExternalInput").ap()
    w1_d = nc.dram_tensor("w1", (EPC, P, NQ, KO, QW), BF16,
                          kind="ExternalInput").ap()
    w2_d = nc.dram_tensor("w2", (EPC, P, SO, DMODEL), BF16,
                          kind="ExternalInput").ap()
    out_d = nc.dram_tensor("out", (EPC, P, MO, DMODEL), FP32,
                           kind="ExternalOutput").ap()

    with tile.TileContext(nc) as tc, ExitStack() as ctx:
        xp = ctx.enter_context(tc.tile_pool(name="xt", bufs=3))
        w1p = ctx.enter_context(tc.tile_pool(name="w1", bufs=2))
        w2p = ctx.enter_context(tc.tile_pool(name="w2", bufs=2))
        hp = ctx.enter_context(tc.tile_pool(name="ht", bufs=2))
        op = ctx.enter_context(tc.tile_pool(name="ot", bufs=2))
        wzp = ctx.enter_context(tc.tile_pool(name="wz", bufs=1))
        ps1 = ctx.enter_context(tc.tile_pool(name="ps1", bufs=6, space="PSUM"))
        ps2 = ctx.enter_context(tc.tile_pool(name="ps2", bufs=2, space="PSUM"))

        # PE warmup: dummy matmuls on zeroed SBUF while the first DMAs land,
        # so HAM un-throttles (1.2 -> 2.4 GHz) before real work — sized to
        # bridge all the way to the first weight chunk's arrival (~14us).
        wz = wzp.tile([P, TOPK], BF16, name="wz", tag="wz")
        nc.any.memzero(wz[:])
        pw = ps2.tile([P, DT], FP32, name="psw", tag="ps2")
        for i in range(30):
            nc.tensor.matmul(pw[:, :TOPK], lhsT=wz[:, :P], rhs=wz[:],
                             start=True, stop=True)

        for e in range(EPC):
            # Loads: all ~0.5-1MB and in consumption order — queues drain
            # round-robin at packet granularity, so uniform chunks keep
            # completion near-FIFO while staying DMA-efficient.
            xt = xp.tile([P, KO, TOPK], BF16, name=f"xt_{e}", tag="xt")
            w1t = w1p.tile([P, NQ, KO, QW], BF16, name=f"w1t_{e}", tag="w1")
            w2t = w2p.tile([P, SO, DMODEL], BF16, name=f"w2t_{e}", tag="w2")
            if e == 0:
                # Split the head of the stream extra-fine so the first
                # matmuls' data lands ASAP (HWDGE completion receipt adds
                # ~3us after the last byte, so small head chunks win).
                nc.sync.dma_start(xt[:, :KO // 2], xt_d[e, :, :KO // 2])
                nc.sync.dma_start(w1t[:, 0, :2], w1_d[e, :, 0, :2])
                nc.sync.dma_start(w1t[:, 0, 2:4], w1_d[e, :, 0, 2:4])
                nc.sync.dma_start(xt[:, KO // 2:], xt_d[e, :, KO // 2:])
                nc.sync.dma_start(w1t[:, 0, 4:6], w1_d[e, :, 0, 4:6])
                nc.sync.dma_start(w1t[:, 0, 6:], w1_d[e, :, 0, 6:])
                for q in range(1, NQ):
                    nc.sync.dma_start(w1t[:, q, :KO // 2],
                                      w1_d[e, :, q, :KO // 2])
                    nc.sync.dma_start(w1t[:, q, KO // 2:],
                                      w1_d[e, :, q, KO // 2:])
            else:
                # Steady-state experts are prefetched a full expert ahead,
                # so fewer/bigger DMAs win (less SP issue time, higher DMA
                # efficiency); round-robin completion skew is absorbed.
                nc.sync.dma_start(xt[:], xt_d[e])
                nc.sync.dma_start(w1t[:, :2], w1_d[e, :, :2])
                nc.sync.dma_start(w1t[:, 2:], w1_d[e, :, 2:])
            if e == 0:
                for s4 in range(SO // 4):
                    nc.sync.dma_start(w2t[:, 4 * s4:4 * s4 + 4],
                                      w2_d[e, :, 4 * s4:4 * s4 + 4])
            else:
                nc.sync.dma_start(w2t[:, :SO // 2], w2_d[e, :, :SO // 2])
                nc.sync.dma_start(w2t[:, SO // 2:], w2_d[e, :, SO // 2:])

            # GEMM1: hT = relu(w1.T @ xT), k-outer over 4 concurrent psum
            # groups per n-quarter (ps1 has 6 slots, so quarter boundaries
            # mostly find free slots and don't wait on the relu drain).
            ht = hp.tile([P, SO * TOPK], BF16, name=f"ht_{e}", tag="ht")
            for q in range(NQ):
                pts = [ps1.tile([P, TOPK], FP32,
                                name=f"ps1_{e}_{q}_{j}", tag="ps1")
                       for j in range(4)]
                for k in range(KO):
                    for j in range(4):
                        nc.tensor.matmul(
                            pts[j][:],
                            lhsT=w1t[:, q, k, j * P:(j + 1) * P],
                            rhs=xt[:, k],
                            start=(k == 0), stop=(k == KO - 1))
                for j in range(4):
                    n0 = q * 4 + j
                    nc.scalar.activation(
                        ht[:, n0 * TOPK:(n0 + 1) * TOPK], pts[j][:], RELU)

            # GEMM2: out = hT.T @ w2, group-sequential over (m,d): staggered
            # completion lets copies/stores overlap remaining matmuls; the
            # last expert stores on the (now idle) HWDGE rings to avoid the
            # SWDGE drain tail.
            ot = op.tile([P, MO, DMODEL], FP32, name=f"ot_{e}", tag="ot")
            last = e == EPC - 1
            for m in range(MO):
                for d in range(ND):
                    final = last and m == MO - 1 and d == ND - 1
                    # The very last group runs as two 256-wide half-groups
                    # (same PE cycles) so its first half's copy+store
                    # overlap the second half's matmuls — shorter tail.
                    widths = [TOPK, TOPK] if final else [DT]
                    off = d * DT
                    for w in widths:
                        pt2g = ps2.tile([P, DT], FP32,
                                        name=f"ps2_{e}_{m}_{d}_{off}",
                                        tag="ps2")
                        for s in range(SO):
                            nc.tensor.matmul(
                                pt2g[:, :w],
                                lhsT=ht[:, s * TOPK + m * P:
                                        s * TOPK + (m + 1) * P],
                                rhs=w2t[:, s, off:off + w],
                                start=(s == 0), stop=(s == SO - 1))
                        nc.vector.tensor_copy(
                            out=ot[:, m, off:off + w], in_=pt2g[:, :w])
                        if last:
                            dma_eng = nc.scalar if d == 0 else nc.sync
                            dma_eng.dma_start(
                                out_d[e, :, m, off:off + w],
                                ot[:, m, off:off + w])
                        off += w
            if not last:
                nc.gpsimd.dma_start(out_d[e], ot[:])

    nc.compile()
    return nc


def _get_nc():
    if "nc" not in _CACHE:
        _CACHE["nc"] = _build_nc()
    return _CACHE["nc"]


def _prep_inputs(x, lin1_weight, lin2_weight):
    bf16 = ml_dtypes.bfloat16
    x_flat = np.asarray(x, dtype=np.float32).reshape(N_EXPERTS * TOPK, DMODEL)
    # xt_all[g, p, ko, m] = x_flat[g*TOPK + m, ko*P + p]
    xt_all = np.ascontiguousarray(
        x_flat.reshape(N_EXPERTS, TOPK, KO, P).transpose(0, 3, 2, 1)
    ).astype(bf16)
    # w1_all[g, p, q, k, c] = lin1[g, k*P + p, q*QW + c]
    w1_all = np.ascontiguousarray(
        np.asarray(lin1_weight, dtype=np.float32)
        .reshape(N_EXPERTS, KO, P, NQ, QW).transpose(0, 2, 3, 1, 4)
    ).astype(bf16)
    # w2_all[g, p, so, d] = lin2[g, so*P + p, d]
    w2_all = np.ascontiguousarray(
        np.asarray(lin2_weight, dtype=np.float32)
        .reshape(N_EXPERTS, SO, P, DMODEL).transpose(0, 2, 1, 3)
    ).astype(bf16)
    in_maps = []
    for c in range(N_CORES):
        sl = slice(c * EPC, (c + 1) * EPC)
        in_maps.append({
            "xt": np.ascontiguousarray(xt_all[sl]),
            "w1": np.ascontiguousarray(w1_all[sl]),
            "w2": np.ascontiguousarray(w2_all[sl]),
        })
    return in_maps


def _ensure_axon_hooks():
    """bass_utils imports antenv.axon_hooks when tracing under axon; the RL
    image ships only an antenv stub. Provide the module (backed by the axon
    PJRT plugin's nrt-profile C ABI when present) so trace requests never
    crash the kernel."""
    try:
        import antenv.axon_hooks  # noqa: F401
        return
    except ImportError:
        pass
    import contextlib
    import ctypes
    import sys
    import types

    hook = None
    so_path = "/opt/axon/libaxon_pjrt.so"
    try:
        lib = ctypes.CDLL(so_path)
        if hasattr(lib, "axon_start_nrt_profile"):
            lib.axon_start_nrt_profile.argtypes = [
                ctypes.POINTER(ctypes.c_int64), ctypes.c_size_t]
            lib.axon_start_nrt_profile.restype = ctypes.c_int64
            lib.axon_stop_nrt_profile.argtypes = [ctypes.c_char_p]
            lib.axon_stop_nrt_profile.restype = ctypes.c_int64

            @contextlib.contextmanager
            def _hook(output_dir, device_ids):
                import jax
                jax.devices()
                if device_ids:
                    ids = (ctypes.c_int64 * len(device_ids))(*device_ids)
                    rc = lib.axon_start_nrt_profile(ids, len(device_ids))
                else:
                    rc = lib.axon_start_nrt_profile(None, 0)
                if rc != 0:
                    raise RuntimeError(f"axon_start_nrt_profile rc={rc}")
                try:
                    yield
                finally:
                    lib.axon_stop_nrt_profile(str(output_dir).encode())

            hook = _hook
    except OSError:
        pass

    mod = types.ModuleType("antenv.axon_hooks")
    mod.get_axon_ntff_profile_hook = lambda: hook
    mod.set_axon_ntff_profile_hook = lambda h: None
    sys.modules["antenv.axon_hooks"] = mod


def _run(in_maps, trace=False, **kw):
    _ensure_axon_hooks()
    from concourse.bass_utils import run_bass_kernel_spmd
    nc = _get_nc()
    return run_bass_kernel_spmd(nc, in_maps, core_ids=list(range(N_CORES)),
                                trace=trace, **kw)


def kernel(x, gate, lin1_weight, lin2_weight, _trace=False, _kw=None):
    in_maps = _prep_inputs(x, lin1_weight, lin2_weight)
    res = _run(in_maps, trace=_trace, **(_kw or {}))
    _CACHE["last_results"] = res
    # res.results[c]["out"]: (EPC, P, MO, DMODEL) fp32
    out_all = np.stack([res.results[c]["out"] for c in range(N_CORES)])
    out_all = out_all.reshape(N_EXPERTS, P, MO, DMODEL)
    # out_e[mo*P + p, d] = out_all[g, p, mo, d]
    out = out_all.transpose(0, 2, 1, 3).reshape(BATCH, CUTOFF, DMODEL)
    return np.ascontiguousarray(out)


# revision 51
# speedup vs baseline: 1.0477x; 1.0038x over previous
"""Expert-choice FF kernel for Trainium2 (Bass/Tile), 8-core expert-parallel.

The reference computes (per expert e of 32):
    xe = x.reshape(32, 256, 1024)[e]          # contiguous token groups
    out[e] = relu(xe @ lin1[e]) @ lin2[e]
(The gate/softmax/top_k in the reference are dead code w.r.t. the output.)

Sharding: expert-parallel, 4 experts per core across 8 cores. Each core runs
two chained GEMMs per expert with fp32 PSUM accumulation over bf16 operands.

Layout (host-prepared so every DMA is contiguous per partition):
    xT[e]  = (128 p, 8 k, 256 m)       : xT[e][p,k,m]   = xe[m, k*128+p]
    w1[e]  = (128 p, 4 q, 8 k, 512 c)  : w1[e][p,q,k,c] = lin1[e][k*128+p, q*512+c]
    w2[e]  = (128 p, 16 s, 1024 d)     : w2[e][p,s,d]   = lin2[e][s*128+p, d]
    out[e] = (128 p, 2 mo, 1024 d)     : out[e][p,mo,d] = out_e[mo*128+p, d]

GEMM1 computes hT (expert_size on partitions) so GEMM2 needs no transpose:
    hT[n,m] = relu(sum_k w1[k,n] * xT[k,m])      (psum [128n, 256m], 8 k-accum)
    out[m,d] = sum_s hT[s,m] * w2[s,d]           (psum [128m, 512d], 16 s-accum)
"""

import numpy as np
import ml_dtypes
from contextlib import ExitStack

N_EXPERTS = 32
TOPK = 256
DMODEL = 1024
EXPERT_SIZE = 2048
BATCH = 2
CUTOFF = 4096

N_CORES = 8
EPC = N_EXPERTS // N_CORES  # experts per core = 4
P = 128
KO = DMODEL // P        # 8 k-tiles (GEMM1 contraction)
SO = EXPERT_SIZE // P   # 16 s-tiles (GEMM2 contraction / GEMM1 output)
MO = TOPK // P          # 2 token chunks
DT = 512                # GEMM2 moving free dim (one psum bank of fp32)
ND = DMODEL // DT       # 2 d-tiles
NQ = 4                  # w1 n-quarters (GEMM1 consumption-ordered chunks)
QW = EXPERT_SIZE // NQ  # 512 n-cols per quarter

_CACHE = {}


def _build_nc():
    import concourse.mybir as mybir
    import concourse.tile as tile
    from concourse import bacc

    BF16 = mybir.dt.bfloat16
    FP32 = mybir.dt.float32
    RELU = mybir.ActivationFunctionType.Relu

    nc = bacc.Bacc("TRN2", debug=False, enable_asserts=False,
                   num_devices=N_CORES, enable_partition_id=False)
    xt_d = nc.dram_tensor("xt", (EPC, P, KO, TOPK), BF16,
                          kind="ExternalInput").ap()
    w1_d = nc.dram_tensor("w1", (EPC, P, NQ, KO, QW), BF16,
                          kind="ExternalInput").ap()
    w2_d = nc.dram_tensor("w2", (EPC, P, SO, DMODEL), BF16,
                          kind="ExternalInput").ap()
    out_d = nc.dram_tensor("out", (EPC, P, MO, DMODEL), FP32,
                           kind="ExternalOutput").ap()

    with tile.TileContext(nc) as tc, ExitStack() as ctx:
        xp = ctx.enter_context(tc.tile_pool(name="xt", bufs=3))
        w1p = ctx.enter_context(tc.tile_pool(name="w1", bufs=2))
        w2p = ctx.enter_context(tc.tile_pool(name="w2", bufs=2))
        hp = ctx.enter_context(tc.tile_pool(name="ht", bufs=2))
        op = ctx.enter_context(tc.tile_pool(name="ot", bufs=2))
        wzp = ctx.enter_context(tc.tile_pool(name="wz", bufs=1))
        ps1 = ctx.enter_context(tc.tile_pool(name="ps1", bufs=6, space="PSUM"))
        ps2 = ctx.enter_context(tc.tile_pool(name="ps2", bufs=2, space="PSUM"))

        # PE warmup: dummy matmuls on zeroed SBUF while the first DMAs land,
        # so HAM un-throttles (1.2 -> 2.4 GHz) before real work — sized to
        # bridge all the way to the first weight chunk's arrival (~14us).
        wz = wzp.tile([P, TOPK], BF16, name="wz", tag="wz")
        nc.any.memzero(wz[:])
        pw = ps2.tile([P, DT], FP32, name="psw", tag="ps2")
        for i in range(32):
            nc.tensor.matmul(pw[:, :TOPK], lhsT=wz[:, :P], rhs=wz[:],
                             start=True, stop=True)

        for e in range(EPC):
            # Loads: all ~0.5-1MB and in consumption order — queues drain
            # round-robin at packet granularity, so uniform chunks keep
            # completion near-FIFO while staying DMA-efficient.
            xt = xp.tile([P, KO, TOPK], BF16, name=f"xt_{e}", tag="xt")
            w1t = w1p.tile([P, NQ, KO, QW], BF16, name=f"w1t_{e}", tag="w1")
            w2t = w2p.tile([P, SO, DMODEL], BF16, name=f"w2t_{e}", tag="w2")
            if e == 0:
                # Split the head of the stream extra-fine so the first
                # matmuls' data lands ASAP (HWDGE completion receipt adds
                # ~3us after the last byte, so small head chunks win).
                nc.sync.dma_start(xt[:, :KO // 2], xt_d[e, :, :KO // 2])
                nc.sync.dma_start(w1t[:, 0, :2], w1_d[e, :, 0, :2])
                nc.sync.dma_start(w1t[:, 0, 2:4], w1_d[e, :, 0, 2:4])
                nc.sync.dma_start(xt[:, KO // 2:], xt_d[e, :, KO // 2:])
                nc.sync.dma_start(w1t[:, 0, 4:6], w1_d[e, :, 0, 4:6])
                nc.sync.dma_start(w1t[:, 0, 6:], w1_d[e, :, 0, 6:])
                for q in range(1, NQ):
                    nc.sync.dma_start(w1t[:, q, :KO // 2],
                                      w1_d[e, :, q, :KO // 2])
                    nc.sync.dma_start(w1t[:, q, KO // 2:],
                                      w1_d[e, :, q, KO // 2:])
            else:
                # Steady-state experts are prefetched a full expert ahead,
                # so fewer/bigger DMAs win (less SP issue time, higher DMA
                # efficiency); round-robin completion skew is absorbed.
                nc.sync.dma_start(xt[:], xt_d[e])
                nc.sync.dma_start(w1t[:, :2], w1_d[e, :, :2])
                nc.sync.dma_start(w1t[:, 2:], w1_d[e, :, 2:])
            if e == 0:
                for s4 in range(SO // 4):
                    nc.sync.dma_start(w2t[:, 4 * s4:4 * s4 + 4],
                                      w2_d[e, :, 4 * s4:4 * s4 + 4])
            else:
                nc.sync.dma_start(w2t[:, :SO // 2], w2_d[e, :, :SO // 2])
                nc.sync.dma_start(w2t[:, SO // 2:], w2_d[e, :, SO // 2:])

            # GEMM1: hT = relu(w1.T @ xT), k-outer over 4 concurrent psum
            # groups per n-quarter (ps1 has 6 slots, so quarter boundaries
            # mostly find free slots and don't wait on the relu drain).
            ht = hp.tile([P, SO * TOPK], BF16, name=f"ht_{e}", tag="ht")
            for q in range(NQ):
                pts = [ps1.tile([P, TOPK], FP32,
                                name=f"ps1_{e}_{q}_{j}", tag="ps1")
                       for j in range(4)]
                for k in range(KO):
                    for j in range(4):
                        nc.tensor.matmul(
                            pts[j][:],
                            lhsT=w1t[:, q, k, j * P:(j + 1) * P],
                            rhs=xt[:, k],
                            start=(k == 0), stop=(k == KO - 1))
                for j in range(4):
                    n0 = q * 4 + j
                    nc.scalar.activation(
                        ht[:, n0 * TOPK:(n0 + 1) * TOPK], pts[j][:], RELU)

            # GEMM2: out = hT.T @ w2, group-sequential over (m,d): staggered
            # completion lets copies/stores overlap remaining matmuls; the
            # last expert stores on the (now idle) HWDGE rings to avoid the
            # SWDGE drain tail.
            ot = op.tile([P, MO, DMODEL], FP32, name=f"ot_{e}", tag="ot")
            last = e == EPC - 1
            for m in range(MO):
                for d in range(ND):
                    final = last and m == MO - 1 and d == ND - 1
                    # The very last group runs as two 256-wide half-groups
                    # (same PE cycles) so its first half's copy+store
                    # overlap the second half's matmuls — shorter tail.
                    widths = [TOPK, TOPK] if final else [DT]
                    off = d * DT
                    for w in widths:
                        pt2g = ps2.tile([P, DT], FP32,
                                        name=f"ps2_{e}_{m}_{d}_{off}",
                                        tag="ps2")
                        for s in range(SO):
                            nc.tensor.matmul(
                                pt2g[:, :w],
                                lhsT=ht[:, s * TOPK + m * P:
                                        s * TOPK + (m + 1) * P],
                                rhs=w2t[:, s, off:off + w],
                                start=(s == 0), stop=(s == SO - 1))
                        nc.vector.tensor_copy(
                            out=ot[:, m, off:off + w], in_=pt2g[:, :w])
                        if last:
                            dma_eng = nc.scalar if d == 0 else nc.sync
                            dma_eng.dma_start(
                                out_d[e, :, m, off:off + w],
                                ot[:, m, off:off + w])
                        off += w
            if not last:
                nc.gpsimd.dma_start(out_d[e], ot[:])

    nc.compile()
    return nc


def _get_nc():
    if "nc" not in _CACHE:
        _CACHE["nc"] = _build_nc()
    return _CACHE["nc"]


def _prep_inputs(x, lin1_weight, lin2_weight):
    bf16 = ml_dtypes.bfloat16
    x_flat = np.asarray(x, dtype=np.float32).reshape(N_EXPERTS * TOPK, DMODEL)
    # xt_all[g, p, ko, m] = x_flat[g*TOPK + m, ko*P + p]
    xt_all = np.ascontiguousarray(
        x_flat.reshape(N_EXPERTS, TOPK, KO, P).transpose(0, 3, 2, 1)
    ).astype(bf16)
    # w1_all[g, p, q, k, c] = lin1[g, k*P + p, q*QW + c]
    w1_all = np.ascontiguousarray(
        np.asarray(lin1_weight, dtype=np.float32)
        .reshape(N_EXPERTS, KO, P, NQ, QW).transpose(0, 2, 3, 1, 4)
    ).astype(bf16)
    # w2_all[g, p, so, d] = lin2[g, so*P + p, d]
    w2_all = np.ascontiguousarray(
        np.asarray(lin2_weight, dtype=np.float32)
        .reshape(N_EXPERTS, SO, P, DMODEL).transpose(0, 2, 1, 3)
    ).astype(bf16)
    in_maps = []
    for c in range(N_CORES):
        sl = slice(c * EPC, (c + 1) * EPC)
        in_maps.append({
            "xt": np.ascontiguousarray(xt_all[sl]),
            "w1": np.ascontiguousarray(w1_all[sl]),
            "w2": np.ascontiguousarray(w2_all[sl]),
        })
    return in_maps


def _ensure_axon_hooks():
    """bass_utils imports antenv.axon_hooks when tracing under axon; the RL
    image ships only an antenv stub. Provide the module (backed by the axon
    PJRT plugin's nrt-profile C ABI when present) so trace requests never
    crash the kernel."""
    try:
        import antenv.axon_hooks  # noqa: F401
        return
    except ImportError:
        pass
    import contextlib
    import ctypes
    import sys
    import types

    hook = None
    so_path = "/opt/axon/libaxon_pjrt.so"
    try:
        lib = ctypes.CDLL(so_path)
        if hasattr(lib, "axon_start_nrt_profile"):
            lib.axon_start_nrt_profile.argtypes = [
                ctypes.POINTER(ctypes.c_int64), ctypes.c_size_t]
            lib.axon_start_nrt_profile.restype = ctypes.c_int64
            lib.axon_stop_nrt_profile.argtypes = [ctypes.c_char_p]
            lib.axon_stop_nrt_profile.restype = ctypes.c_int64

            @contextlib.contextmanager
            def _hook(output_dir, device_ids):
                import jax
                jax.devices()
                if device_ids:
                    ids = (ctypes.c_int64 * len(device_ids))(*device_ids)
                    rc = lib.axon_start_nrt_profile(ids, len(device_ids))
                else:
                    rc = lib.axon_start_nrt_profile(None, 0)
                if rc != 0:
                    raise RuntimeError(f"axon_start_nrt_profile rc={rc}")
                try:
                    yield
                finally:
                    lib.axon_stop_nrt_profile(str(output_dir).encode())

            hook = _hook
    except OSError:
        pass

    mod = types.ModuleType("antenv.axon_hooks")
    mod.get_axon_ntff_profile_hook = lambda: hook
    mod.set_axon_ntff_profile_hook = lambda h: None
    sys.modules["antenv.axon_hooks"] = mod


def _run(in_maps, trace=False, **kw):
    _ensure_axon_hooks()
    from concourse.bass_utils import run_bass_kernel_spmd
    nc = _get_nc()
    return run_bass_kernel_spmd(nc, in_maps, core_ids=list(range(N_CORES)),
                                trace=trace, **kw)


def kernel(x, gate, lin1_weight, lin2_weight, _trace=False, _kw=None):
    in_maps = _prep_inputs(x, lin1_weight, lin2_weight)
    res = _run(in_maps, trace=_trace, **(_kw or {}))
    _CACHE["last_results"] = res
    # res.results[c]["out"]: (EPC, P, MO, DMODEL) fp32
    out_all = np.stack([res.results[c]["out"] for c in range(N_CORES)])
    out_all = out_all.reshape(N_EXPERTS, P, MO, DMODEL)
    # out_e[mo*P + p, d] = out_all[g, p, mo, d]
    out = out_all.transpose(0, 2, 1, 3).reshape(BATCH, CUTOFF, DMODEL)
    return np.ascontiguousarray(out)
